# revision 51
# baseline (speedup 1.0000x reference)
"""Trainium2 Bass kernel for nn_Brain (encoder MLP -> bidirectional LSTM -> decoder MLP).

Sharding: data-parallel over N=1024 batch across 8 cores (n=128 each); small
weights replicated; the T=512 recurrence runs locally per core as two
interleaved chains (forward + backward), chain f staggered half a period
ahead of chain b.

Device layout (per core): channels on the 128 SBUF partitions, (time, batch)
on the free axis, so the LSTM state h^T [H=128, n=128] feeds the per-gate
matmuls (stationary bf16 gate weights, 1 cycle/row) with no transposes.

LSTM step (per chain), gate order f,i,g,o with the g-gate weights
pre-doubled on the host so tanh(g) = 2*sigmoid(2g) - 1:
  ACT: S[0:3N] = sigmoid(G[f,i,g2])     one op covers all of the c-path
  ACT: S[3N:4N] = sigmoid(G[o])         floated (no ring edge), fills gaps
  DVE: p  = 2*S[g] - 1                  TensorScalar, bf16 4x mode (93ns)
  DVE: M  = S[f,i] * [c|p]              fused 2N TensorTensor, 2x (193ns)
  DVE: cn = M[f] + M[i]
  ACT: tc = tanh(cn)
  DVE: h  = S[o] * tc
All elementwise tiles bf16 in SBUF (DVE 2x/4x perf modes), c included
(validated: adds ~nothing over f32-c at final rel err 4.6e-3 << 2e-2).
Same-engine "ring" nosync deps order the two chains' ACT ops so their
serial chains hide each other (acts ring only; ves ring hurts).

The encoder runs INSIDE the recurrence's first half (chunks emitted from
both time-ends inward; features land in an SBUF-resident [H, T*n] bf16
tile - no DRAM roundtrip, and the x-part matmuls read it directly). The
decoder runs inside the second half (middle-out chunk order, as soon as
both directions' h are flushed to DRAM): h-relus on the otherwise idle
Pool engine, PSUM-sourced bias+relu/adds split into 256-col pieces and
slotted into measured ACT/DVE idle gaps (pinned with nosync deps so the
out-of-order engine windows can't hoist them onto the critical path).
Output rows batch in SBUF (4 chunks per DMA). GPSIMD cannot access PSUM
and DMA cannot read PSUM, so every PSUM egress goes through ACT or DVE.
"""
import sys

sys.path.insert(0, "/opt/trn_rl_repo")

import numpy as np

import concourse.bass as bass
import concourse.bacc as bacc
import concourse.tile as tile
from concourse import mybir
from concourse.bass_utils import run_bass_kernel_spmd

F32 = mybir.dt.float32
F32R = mybir.dt.float32r
BF16 = mybir.dt.bfloat16
AF = mybir.ActivationFunctionType
ALU = mybir.AluOpType
CDT = BF16  # dtype of the c-chain tiles
RING = "acts"  # ring-order deps: both | acts | ves | none
ALT = False  # alternate which chain leads each step
FLOATO = True  # leave sigof(A) out of the acts ring

N, T, P, H = 1024, 512, 2, 128
NCORES = 8
NPC = N // NCORES  # 128 batch rows per core

LAST_RESULTS = None  # set by kernel(); test.py reads profiling info from here


def build_program(tsteps=T, dbg=False):
    nc = bacc.Bacc("TRN2", target_bir_lowering=False, debug=False, num_devices=NCORES)
    tn = tsteps * NPC

    xt = nc.declare_dram_parameter("xt", [P, tn], F32R, isOutput=False)
    wstack = nc.declare_dram_parameter("wstack", [H, 8 * H], F32, isOutput=False)
    e1wT = nc.declare_dram_parameter("e1wT", [P, H], F32R, isOutput=False)
    e1b = nc.declare_dram_parameter("e1b", [H, 1], F32, isOutput=False)
    e2wT = nc.declare_dram_parameter("e2wT", [H, H], F32, isOutput=False)
    e2b = nc.declare_dram_parameter("e2b", [H, 1], F32, isOutput=False)
    dwT = nc.declare_dram_parameter("dwT", [H, 2 * H], F32, isOutput=False)
    d1b = nc.declare_dram_parameter("d1b", [H, 1], F32, isOutput=False)
    d2T = nc.declare_dram_parameter("d2T", [H, 1], F32, isOutput=False)
    d2b = nc.declare_dram_parameter("d2b", [1, 1], F32, isOutput=False)
    out = nc.declare_dram_parameter("out", [tn // 1024, 1024], F32, isOutput=True)

    if dbg:
        featbuf = nc.declare_dram_parameter("featbuf", [H, tn], BF16, isOutput=True)
        hfwbuf = nc.declare_dram_parameter("hfwbuf", [H, tn], BF16, isOutput=True)
        hbwbuf = nc.declare_dram_parameter("hbwbuf", [H, tn], BF16, isOutput=True)
    else:
        hfwbuf = nc.dram_tensor("hfwbuf", [H, tn], BF16)
        hbwbuf = nc.dram_tensor("hbwbuf", [H, tn], BF16)

    ECH = 512  # encoder/decoder chunk: 4 timesteps of (t,n) columns
    nchunk = tn // ECH  # = tsteps // 4
    HG = 4  # h-state flush group (steps per DMA)

    with tile.TileContext(nc) as tc:
        with tc.tile_pool(name="singles", bufs=1) as singles:
            w_sb = singles.tile([H, 8 * H], F32)
            nc.sync.dma_start(w_sb[:], wstack[:])
            e1w_sb = singles.tile([P, H], F32R)
            nc.sync.dma_start(e1w_sb[:], e1wT[:])
            e1b_sb = singles.tile([H, 1], F32)
            nc.sync.dma_start(e1b_sb[:], e1b[:])
            e2w_sb = singles.tile([H, H], F32)
            nc.sync.dma_start(e2w_sb[:], e2wT[:])
            e2b_sb = singles.tile([H, 1], F32)
            nc.sync.dma_start(e2b_sb[:], e2b[:])
            dw_sb = singles.tile([H, 2 * H], F32)
            nc.sync.dma_start(dw_sb[:], dwT[:])
            d1b_sb = singles.tile([H, 1], F32)
            nc.sync.dma_start(d1b_sb[:], d1b[:])
            d2T_sb = singles.tile([H, 1], F32)
            nc.sync.dma_start(d2T_sb[:], d2T[:])
            d2b_sb = singles.tile([1, 1], F32)
            nc.sync.dma_start(d2b_sb[:], d2b[:])
            # bf16 copies of all recurrent/decoder weights (1 cycle/row matmuls)
            wx_bf = singles.tile([H, 4 * H], BF16)
            nc.vector.tensor_copy(wx_bf[:], w_sb[:, 0 : 4 * H])
            u_bf = singles.tile([H, 4 * H], BF16)
            nc.vector.tensor_copy(u_bf[:], w_sb[:, 4 * H : 8 * H])
            dw_bf = singles.tile([H, 2 * H], BF16)
            nc.vector.tensor_copy(dw_bf[:], dw_sb[:])
            e2w_bf = singles.tile([H, H], BF16)
            nc.vector.tensor_copy(e2w_bf[:], e2w_sb[:])
            d2_bf = singles.tile([H, 1], BF16)
            nc.vector.tensor_copy(d2_bf[:], d2T_sb[:])

            # SBUF-resident encoder output (bf16): [H, tsteps*NPC]
            featSB = singles.tile([H, tn], BF16)

            with (
                tc.tile_pool(name="encio", bufs=3) as encio,
                tc.tile_pool(name="decio", bufs=3) as decio,
                tc.tile_pool(name="orow", bufs=1) as orowp,
                tc.tile_pool(name="state", bufs=3) as state,
                tc.tile_pool(name="relem", bufs=2) as relem,
                tc.tile_pool(name="rpsum", bufs=2, space="PSUM") as rpsum,
                tc.tile_pool(name="auxp", bufs=2, space="PSUM") as auxp,
            ):

                def enc_chunk(j, slots=None):
                    # stage A (immediate): x DMA + first matmul
                    # slot stages: a1 relu (DVE, PSUM source) + mm2; ft add (DVE)
                    sl = slice(j * ECH, (j + 1) * ECH)
                    xtile = encio.tile([P, ECH], F32R, tag="xtile")
                    nc.sync.dma_start(xtile[:], xt[:, sl])
                    ps1 = auxp.tile([H, ECH], F32, tag="pA")
                    nc.tensor.matmul(ps1[:], e1w_sb[:], xtile[:], start=True, stop=True)

                    EH = ECH // 2
                    box = {}

                    def stage_a1(half):
                        if half == 0:
                            box["a1"] = encio.tile([H, ECH], BF16, tag="a1", name="a1")
                        hs = slice(half * EH, (half + 1) * EH)
                        op = nc.vector.tensor_scalar(
                            out=box["a1"][:, hs], in0=ps1[:, hs], scalar1=e1b_sb[:, 0:1],
                            scalar2=0.0, op0=ALU.add, op1=ALU.max,
                        )
                        if half == 1:
                            ps2 = auxp.tile([H, ECH], F32, tag="pB", name="ps2")
                            nc.tensor.matmul(ps2[:], e2w_bf[:], box["a1"][:], start=True, stop=True)
                            box["ps2"] = ps2
                        return op

                    def stage_ft(half):
                        hs = slice(half * EH, (half + 1) * EH)
                        fsl = slice(j * ECH + half * EH, j * ECH + (half + 1) * EH)
                        op = nc.vector.tensor_scalar(
                            out=featSB[:, fsl], in0=box["ps2"][:, hs], scalar1=e2b_sb[:, 0:1],
                            scalar2=None, op0=ALU.add,
                        )
                        if dbg and half == 1:
                            nc.sync.dma_start(featbuf[:, sl], featSB[:, sl])
                        return op

                    if slots is None:
                        # warmup/tail: ACT is idle here, use one fused op per layer
                        a1 = encio.tile([H, ECH], BF16, tag="a1")
                        nc.scalar.activation(a1[:], ps1[:], AF.Relu, bias=e1b_sb[:, 0:1])
                        ps2 = auxp.tile([H, ECH], F32, tag="pB")
                        nc.tensor.matmul(ps2[:], e2w_bf[:], a1[:], start=True, stop=True)
                        nc.vector.tensor_scalar(
                            out=featSB[:, sl], in0=ps2[:], scalar1=e2b_sb[:, 0:1],
                            scalar2=None, op0=ALU.add,
                        )
                        if dbg:
                            nc.sync.dma_start(featbuf[:, sl], featSB[:, sl])
                    else:
                        slots.extend([lambda: stage_a1(0), lambda: stage_a1(1),
                                      lambda: stage_ft(0), lambda: stage_ft(1)])

                OW = 4  # decoder chunks batched per output DMA
                owpos = {}  # side -> (tile, count, first_row)

                def dec_chunk(j, side, slots=None):
                    # immediate: h DMAs + Pool relus + psQ matmuls
                    # slot stages: q1 relu (DVE); d2 matmul + orow add (DVE)
                    sl = slice(j * ECH, (j + 1) * ECH)
                    hfr = decio.tile([H, ECH], BF16, tag="hfr")
                    nc.sync.dma_start(hfr[:], hfwbuf[:, sl])
                    hbr = decio.tile([H, ECH], BF16, tag="hbr")
                    nc.sync.dma_start(hbr[:], hbwbuf[:, sl])
                    hf_t = decio.tile([H, ECH], BF16, tag="hf")
                    hb_t = decio.tile([H, ECH], BF16, tag="hb")
                    if slots is None:
                        nc.scalar.activation(hf_t[:], hfr[:], AF.Relu)
                    else:
                        nc.gpsimd.tensor_scalar(
                            out=hf_t[:], in0=hfr[:], scalar1=0.0, scalar2=None, op0=ALU.max
                        )
                    nc.gpsimd.tensor_scalar(
                        out=hb_t[:], in0=hbr[:], scalar1=0.0, scalar2=None, op0=ALU.max
                    )
                    psQ = auxp.tile([H, ECH], F32, tag="pA")
                    nc.tensor.matmul(psQ[:], dw_bf[:, 0:H], hf_t[:], start=True, stop=False)
                    nc.tensor.matmul(psQ[:], dw_bf[:, H : 2 * H], hb_t[:], start=False, stop=True)

                    EH = ECH // 2
                    box = {}

                    def stage_q1(half):
                        if half == 0:
                            box["q1"] = decio.tile([H, ECH], BF16, tag="q1", name="q1")
                        hs = slice(half * EH, (half + 1) * EH)
                        op = nc.scalar.activation(
                            box["q1"][:, hs], psQ[:, hs], AF.Relu, bias=d1b_sb[:, 0:1]
                        )
                        if half == 1:
                            psR = auxp.tile([H, ECH], F32, tag="pB", name="psR")
                            nc.tensor.matmul(psR[0:1, :], d2_bf[:], box["q1"][:], start=True, stop=True)
                            box["psR"] = psR
                        return op

                    def stage_orow(half):
                        ow = owpos.get(side)
                        if ow is None:
                            owt = orowp.tile(
                                [1, OW * ECH], F32, tag=f"ow{side}", name=f"ow{side}"
                            )
                            ow = [owt, 0, j]
                            owpos[side] = ow
                        tilebuf, cnt, j0 = ow
                        pos = (j - j0) * ECH if side == "R" else (j - (j0 - (OW - 1))) * ECH
                        hs = slice(half * EH, (half + 1) * EH)
                        op = nc.vector.tensor_scalar(
                            out=tilebuf[0:1, pos + half * EH : pos + (half + 1) * EH],
                            in0=box["psR"][0:1, hs],
                            scalar1=d2b_sb[0:1, 0:1], scalar2=None, op0=ALU.add,
                        )
                        if half == 1:
                            ow[1] += 1
                            if ow[1] == OW:
                                first = j0 if side == "R" else j0 - (OW - 1)
                                r0 = first * ECH // 1024
                                nrows = OW * ECH // 1024
                                nc.sync.dma_start(out[r0 : r0 + nrows, :], tilebuf[:])
                                del owpos[side]
                        return op

                    if slots is None:
                        stage_q1(0)
                        stage_q1(1)
                        stage_orow(0)
                        stage_orow(1)
                    else:
                        def stage_q1b_then_orow():
                            # defer orow pieces until psR exists
                            op = stage_q1(1)
                            slots.extend([lambda: stage_orow(0), lambda: stage_orow(1)])
                            return op

                        aslots.extend([lambda: stage_q1(0), stage_q1b_then_orow])

                # ---- encoder warmup: both time-ends, 4 chunks each ----
                npre = min(5, nchunk // 2)
                for j in range(npre):
                    enc_chunk(j)
                    enc_chunk(nchunk - 1 - j)

                # ---------------- bidirectional LSTM recurrence ----------------
                def xstep(ch, t, close=False):
                    tt = t if ch == "f" else tsteps - 1 - t
                    g = rpsum.tile([H, 4 * NPC], F32, tag=f"G{ch}", name=f"G{ch}t")
                    for gi in range(4):
                        nc.tensor.matmul(
                            g[:, gi * NPC : (gi + 1) * NPC],
                            wx_bf[:, gi * H : (gi + 1) * H],
                            featSB[:, tt * NPC : (tt + 1) * NPC],
                            start=(gi == 0),
                            stop=close and gi == 3,
                        )
                    return g

                G = {ch: xstep(ch, 0, close=True) for ch in ("f", "b")}
                Gnext = {}
                h = {ch: None for ch in ("f", "b")}
                CP = {}  # [c | p] tile per chain
                hpar = {}
                for ch in ("f", "b"):
                    CP[ch] = state.tile([H, 2 * NPC], CDT, tag=f"CP{ch}", name=f"CP{ch}0")
                    nc.vector.memset(CP[ch][:, 0:NPC], 0.0)

                DEC0 = tsteps // 2 + 12
                slots = []  # staged enc/dec bulk-DVE thunks
                aslots = []  # staged dec ACT thunks (popped in the sigob->tcb window)
                for t in range(tsteps):
                    # PE: h-parts for step t, then x-parts for step t+1
                    for ch in ("f", "b"):
                        if h[ch] is not None:
                            for gi in range(4):
                                nc.tensor.matmul(
                                    G[ch][:, gi * NPC : (gi + 1) * NPC],
                                    u_bf[:, gi * H : (gi + 1) * H],
                                    h[ch][:],
                                    start=False,
                                    stop=(gi == 3),
                                    skip_group_check=True,
                                )
                        if t + 1 < tsteps:
                            Gnext[ch] = xstep(ch, t + 1)
                    # elementwise, forced same-engine ring order tuned so the
                    # staggered chains hide each other's serial latency:
                    #   ACT: sig3f sigof sig3b tcf sigob tcb
                    #   DVE: pf mm12f addf pb mm12b hnf addb hnb
                    acts = []
                    ves = []
                    S = {}
                    CPn = {}
                    M = {}
                    tcn = {}
                    hns = {}

                    def sig3(ch):
                        S[ch] = relem.tile([H, 4 * NPC], BF16, tag=f"S{ch}", name=f"S{ch}t")
                        acts.append(
                            nc.scalar.activation(
                                S[ch][:, 0 : 3 * NPC], G[ch][:, 0 : 3 * NPC], AF.Sigmoid
                            )
                        )

                    def sigo(ch):
                        acts.append(
                            nc.scalar.activation(
                                S[ch][:, 3 * NPC : 4 * NPC],
                                G[ch][:, 3 * NPC : 4 * NPC],
                                AF.Sigmoid,
                            )
                        )

                    def cblock(ch):
                        # p = 2*s_g - 1 into CP right half (4x TensorScalar)
                        ves.append(
                            nc.vector.tensor_scalar(
                                out=CP[ch][:, NPC : 2 * NPC],
                                in0=S[ch][:, 2 * NPC : 3 * NPC],
                                scalar1=2.0,
                                scalar2=-1.0,
                                op0=ALU.mult,
                                op1=ALU.add,
                            )
                        )
                        M[ch] = relem.tile([H, 2 * NPC], CDT, tag=f"M{ch}", name=f"M{ch}t")
                        ves.append(
                            nc.vector.tensor_mul(M[ch][:], S[ch][:, 0 : 2 * NPC], CP[ch][:])
                        )

                    def cadd(ch):
                        CPn[ch] = state.tile([H, 2 * NPC], CDT, tag=f"CP{ch}", name=f"CP{ch}n")
                        ves.append(
                            nc.vector.tensor_add(
                                CPn[ch][:, 0:NPC], M[ch][:, 0:NPC], M[ch][:, NPC : 2 * NPC]
                            )
                        )

                    def tanhc(ch):
                        tcn[ch] = relem.tile([H, NPC], CDT, tag=f"tc{ch}", name=f"tc{ch}t")
                        acts.append(nc.scalar.activation(tcn[ch][:], CPn[ch][:, 0:NPC], AF.Tanh))

                    def hmul(ch):
                        if t % HG == 0:
                            hpar[ch] = state.tile(
                                [H, HG * NPC], BF16, tag=f"h{ch}", name=f"h{ch}n"
                            )
                        sb = t % HG if ch == "f" else HG - 1 - (t % HG)
                        hn = hpar[ch][:, sb * NPC : (sb + 1) * NPC]
                        hns[ch] = hn
                        ves.append(
                            nc.vector.tensor_mul(hn, S[ch][:, 3 * NPC : 4 * NPC], tcn[ch][:])
                        )

                    A, B = ("f", "b") if t % 2 == 0 or not ALT else ("b", "f")
                    sig3(A)
                    sigo(A)
                    sig3(B)
                    cblock(A)
                    cadd(A)
                    tanhc(A)
                    cblock(B)
                    sigo(B)
                    hmul(A)
                    cadd(B)
                    if slots:
                        op = slots.pop(0)()  # bulk piece in the addb->hnb DVE gap
                        tile.add_dep_helper(op.ins, ves[-1].ins, sync=False, reason="slot pin")
                    tanhc(B)
                    hmul(B)
                    if aslots:
                        op = aslots.pop(0)()  # dec ACT piece in the period-end ACT idle
                        tile.add_dep_helper(op.ins, acts[-1].ins, sync=False, reason="slot pin")
                    if slots:
                        op = slots.pop(0)()  # second piece in the post-hnb DVE gap
                        tile.add_dep_helper(op.ins, ves[-1].ins, sync=False, reason="slot pin")
                    for ch in ("f", "b"):
                        if t % HG == HG - 1:
                            dst = hfwbuf if ch == "f" else hbwbuf
                            lo = (t - HG + 1) if ch == "f" else (tsteps - 1 - t)
                            nc.sync.dma_start(dst[:, lo * NPC : (lo + HG) * NPC], hpar[ch][:])
                        h[ch] = hns[ch]
                        CP[ch] = CPn[ch]
                    rsrc = {"both": (acts, ves), "acts": (acts,), "ves": (ves,), "none": ()}[RING]
                    for seq in rsrc:
                        ops = [o for o in seq if o is not acts[1]] if (seq is acts and FLOATO) else seq
                        for a, b2 in zip(ops, ops[1:]):
                            tile.add_dep_helper(b2.ins, a.ins, sync=False, reason="ring order")
                    G = Gnext
                    Gnext = {}
                    # interleaved encoder (first half) / decoder (second half)
                    if t % 4 == 0:
                        k = t // 4
                        if npre + k < nchunk // 2:
                            enc_chunk(npre + k, slots)
                            enc_chunk(nchunk - 1 - npre - k, slots)
                    if t >= DEC0 and t % 2 == 0:
                        m = (t - DEC0) // 4
                        if nchunk // 2 + m < nchunk - 4:
                            if t % 4 == 0:
                                dec_chunk(nchunk // 2 + m, "R", slots)
                            else:
                                dec_chunk(nchunk // 2 - 1 - m, "L", slots)

                for th in aslots + slots:
                    th()
                slots = []
                aslots = []

                # ---- decoder tail: remaining chunks ----
                done = set()
                if DEC0 % 4 != 0:
                    DEC0 += 4 - DEC0 % 4
                for t in range(DEC0, tsteps, 4):
                    m = (t - DEC0) // 4
                    if nchunk // 2 + m < nchunk - 4:
                        done.add(nchunk // 2 + m)
                        done.add(nchunk // 2 - 1 - m)
                rest = sorted(set(range(nchunk)) - done)
                # emit leftovers from both ends inward, in OW-aligned groups
                lo_side = sorted([j for j in rest if j < nchunk // 2], reverse=True)
                hi_side = sorted([j for j in rest if j >= nchunk // 2])
                for jl, jr in zip(lo_side, hi_side):
                    dec_chunk(jl, "L")
                    dec_chunk(jr, "R")
                for j in lo_side[len(hi_side):]:
                    dec_chunk(j, "L")
                for j in hi_side[len(lo_side):]:
                    dec_chunk(j, "R")

    nc.finalize()
    return nc


def _prep_shared(inputs):
    w_ih, w_hh = inputs["w_ih"], inputs["w_hh"]
    blocks = lambda w: (w[0:H], w[H : 2 * H], w[2 * H : 3 * H], w[3 * H : 4 * H])
    Wi, Wf, Wg, Wo = blocks(w_ih)
    Ui, Uf, Ug, Uo = blocks(w_hh)
    f32 = lambda a: np.ascontiguousarray(a, dtype=np.float32)
    shared = {
        # gate order f, i, g, o; g-gate weights doubled so
        # tanh(g) = 2*sigmoid(2g) - 1
        "wstack": f32(
            np.concatenate(
                [Wf.T, Wi.T, 2.0 * Wg.T, Wo.T, Uf.T, Ui.T, 2.0 * Ug.T, Uo.T], axis=1
            )
        ),
        "e1wT": f32(inputs["enc1_w"].T),
        "e1b": f32(inputs["enc1_b"][:, None]),
        "e2wT": f32(inputs["enc2_w"].T),
        "e2b": f32(inputs["enc2_b"][:, None]),
        "dwT": f32(
            np.concatenate([inputs["dec1_w"][:, :H].T, inputs["dec1_w"][:, H:].T], axis=1)
        ),
        "d1b": f32(inputs["dec1_b"][:, None]),
        "d2T": f32(inputs["dec2_w"].T),
        "d2b": f32(inputs["dec2_b"][:, None]),
    }
    return shared


_NC_CACHE = None


def _get_nc():
    global _NC_CACHE
    if _NC_CACHE is None:
        _NC_CACHE = build_program(T)
    return _NC_CACHE


def kernel(**inputs) -> np.ndarray:
    global LAST_RESULTS
    inputs = {k: np.asarray(v) for k, v in inputs.items()}
    x = inputs["x"]

    nc = _get_nc()
    shared = _prep_shared(inputs)

    in_maps = []
    for cidx in range(NCORES):
        xc = x[cidx * NPC : (cidx + 1) * NPC]  # (128, T, 2)
        xtc = np.ascontiguousarray(
            xc.transpose(2, 1, 0).reshape(P, T * NPC), dtype=np.float32
        )
        m = {"xt": xtc}
        m.update(shared)
        in_maps.append(m)

    res = run_bass_kernel_spmd(nc, in_maps, core_ids=list(range(NCORES)))
    LAST_RESULTS = res

    outs = []
    for cidx in range(NCORES):
        o = res.results[cidx]["out"]  # [T//8, 1024]; o[j, k*128+n] = q[n, 8j+k]
        q = o.reshape(T // 8, 8, NPC).transpose(2, 0, 1).reshape(NPC, T)
        outs.append(q)
    return np.ascontiguousarray(np.concatenate(outs, axis=0), dtype=np.float32)


if __name__ == "__main__":
    print("building program...")
    nc = build_program(32)
    print("ok, instructions:", sum(len(bb.instructions) for bb in nc.main_func.blocks))


# revision 52
# speedup vs baseline: 1.0111x; 1.0111x over previous
"""Trainium2 Bass kernel for nn_Brain (encoder MLP -> bidirectional LSTM -> decoder MLP).

Sharding: data-parallel over N=1024 batch across 8 cores (n=128 each); small
weights replicated; the T=512 recurrence runs locally per core as two
interleaved chains (forward + backward), chain f staggered half a period
ahead of chain b.

Device layout (per core): channels on the 128 SBUF partitions, (time, batch)
on the free axis, so the LSTM state h^T [H=128, n=128] feeds the per-gate
matmuls (stationary bf16 gate weights, 1 cycle/row) with no transposes.

LSTM step (per chain), gate order f,i,g,o with the g-gate weights
pre-doubled on the host so tanh(g) = 2*sigmoid(2g) - 1:
  ACT: S[0:3N] = sigmoid(G[f,i,g2])     one op covers all of the c-path
  ACT: S[3N:4N] = sigmoid(G[o])         floated (no ring edge), fills gaps
  DVE: p  = 2*S[g] - 1                  TensorScalar, bf16 4x mode (93ns)
  DVE: M  = S[f,i] * [c|p]              fused 2N TensorTensor, 2x (193ns)
  DVE: cn = M[f] + M[i]
  ACT: tc = tanh(cn)
  DVE: h  = S[o] * tc
All elementwise tiles bf16 in SBUF (DVE 2x/4x perf modes), c included
(validated: adds ~nothing over f32-c at final rel err 4.6e-3 << 2e-2).
Same-engine "ring" nosync deps order the two chains' ACT ops so their
serial chains hide each other (acts ring only; ves ring hurts).

The encoder runs INSIDE the recurrence's first half (chunks emitted from
both time-ends inward; features land in an SBUF-resident [H, T*n] bf16
tile - no DRAM roundtrip, and the x-part matmuls read it directly). The
decoder runs inside the second half (middle-out chunk order, as soon as
both directions' h are flushed to DRAM): h-relus on the otherwise idle
Pool engine, PSUM-sourced bias+relu/adds split into 256-col pieces and
slotted into measured ACT/DVE idle gaps (pinned with nosync deps so the
out-of-order engine windows can't hoist them onto the critical path).
Output rows batch in SBUF (4 chunks per DMA). GPSIMD cannot access PSUM
and DMA cannot read PSUM, so every PSUM egress goes through ACT or DVE.
"""
import sys

sys.path.insert(0, "/opt/trn_rl_repo")

import numpy as np

import concourse.bass as bass
import concourse.bacc as bacc
import concourse.tile as tile
from concourse import mybir
from concourse.bass_utils import run_bass_kernel_spmd

F32 = mybir.dt.float32
F32R = mybir.dt.float32r
BF16 = mybir.dt.bfloat16
AF = mybir.ActivationFunctionType
ALU = mybir.AluOpType
CDT = BF16  # dtype of the c-chain tiles
RING = "acts"  # ring-order deps: both | acts | ves | none
ALT = False  # alternate which chain leads each step
FLOATO = True  # leave sigof(A) out of the acts ring

N, T, P, H = 1024, 512, 2, 128
NCORES = 8
NPC = N // NCORES  # 128 batch rows per core

LAST_RESULTS = None  # set by kernel(); test.py reads profiling info from here


def build_program(tsteps=T, dbg=False):
    nc = bacc.Bacc("TRN2", target_bir_lowering=False, debug=False, num_devices=NCORES)
    tn = tsteps * NPC

    xt = nc.declare_dram_parameter("xt", [P, tn], F32R, isOutput=False)
    wstack = nc.declare_dram_parameter("wstack", [H, 8 * H], F32, isOutput=False)
    e1wT = nc.declare_dram_parameter("e1wT", [P, H], F32R, isOutput=False)
    e1b = nc.declare_dram_parameter("e1b", [H, 1], F32, isOutput=False)
    e2wT = nc.declare_dram_parameter("e2wT", [H, H], F32, isOutput=False)
    e2b = nc.declare_dram_parameter("e2b", [H, 1], F32, isOutput=False)
    dwT = nc.declare_dram_parameter("dwT", [H, 2 * H], F32, isOutput=False)
    d1b = nc.declare_dram_parameter("d1b", [H, 1], F32, isOutput=False)
    d2T = nc.declare_dram_parameter("d2T", [H, 1], F32, isOutput=False)
    d2b = nc.declare_dram_parameter("d2b", [1, 1], F32, isOutput=False)
    out = nc.declare_dram_parameter("out", [tn // 1024, 1024], F32, isOutput=True)

    if dbg:
        featbuf = nc.declare_dram_parameter("featbuf", [H, tn], BF16, isOutput=True)
        hfwbuf = nc.declare_dram_parameter("hfwbuf", [H, tn], BF16, isOutput=True)
        hbwbuf = nc.declare_dram_parameter("hbwbuf", [H, tn], BF16, isOutput=True)
    else:
        hfwbuf = nc.dram_tensor("hfwbuf", [H, tn], BF16)
        hbwbuf = nc.dram_tensor("hbwbuf", [H, tn], BF16)

    ECH = 512  # encoder/decoder chunk: 4 timesteps of (t,n) columns
    nchunk = tn // ECH  # = tsteps // 4
    HG = 4  # h-state flush group (steps per DMA)

    with tile.TileContext(nc) as tc:
        with tc.tile_pool(name="singles", bufs=1) as singles:
            w_sb = singles.tile([H, 8 * H], F32)
            nc.sync.dma_start(w_sb[:], wstack[:])
            e1w_sb = singles.tile([P, H], F32R)
            nc.sync.dma_start(e1w_sb[:], e1wT[:])
            e1b_sb = singles.tile([H, 1], F32)
            nc.sync.dma_start(e1b_sb[:], e1b[:])
            e2w_sb = singles.tile([H, H], F32)
            nc.sync.dma_start(e2w_sb[:], e2wT[:])
            e2b_sb = singles.tile([H, 1], F32)
            nc.sync.dma_start(e2b_sb[:], e2b[:])
            dw_sb = singles.tile([H, 2 * H], F32)
            nc.sync.dma_start(dw_sb[:], dwT[:])
            d1b_sb = singles.tile([H, 1], F32)
            nc.sync.dma_start(d1b_sb[:], d1b[:])
            d2T_sb = singles.tile([H, 1], F32)
            nc.sync.dma_start(d2T_sb[:], d2T[:])
            d2b_sb = singles.tile([1, 1], F32)
            nc.sync.dma_start(d2b_sb[:], d2b[:])
            # bf16 copies of all recurrent/decoder weights (1 cycle/row matmuls)
            wx_bf = singles.tile([H, 4 * H], BF16)
            nc.vector.tensor_copy(wx_bf[:], w_sb[:, 0 : 4 * H])
            u_bf = singles.tile([H, 4 * H], BF16)
            nc.vector.tensor_copy(u_bf[:], w_sb[:, 4 * H : 8 * H])
            dw_bf = singles.tile([H, 2 * H], BF16)
            nc.vector.tensor_copy(dw_bf[:], dw_sb[:])
            e2w_bf = singles.tile([H, H], BF16)
            nc.vector.tensor_copy(e2w_bf[:], e2w_sb[:])
            d2_bf = singles.tile([H, 1], BF16)
            nc.vector.tensor_copy(d2_bf[:], d2T_sb[:])

            # SBUF-resident encoder output (bf16): [H, tsteps*NPC]
            featSB = singles.tile([H, tn], BF16)

            with (
                tc.tile_pool(name="encio", bufs=3) as encio,
                tc.tile_pool(name="decio", bufs=3) as decio,
                tc.tile_pool(name="orow", bufs=1) as orowp,
                tc.tile_pool(name="state", bufs=3) as state,
                tc.tile_pool(name="relem", bufs=2) as relem,
                tc.tile_pool(name="rpsum", bufs=2, space="PSUM") as rpsum,
                tc.tile_pool(name="auxp", bufs=2, space="PSUM") as auxp,
            ):

                def enc_chunk(j, slots=None):
                    # stage A (immediate): x DMA + first matmul
                    # slot stages: a1 relu (DVE, PSUM source) + mm2; ft add (DVE)
                    sl = slice(j * ECH, (j + 1) * ECH)
                    xtile = encio.tile([P, ECH], F32R, tag="xtile")
                    nc.sync.dma_start(xtile[:], xt[:, sl])
                    ps1 = auxp.tile([H, ECH], F32, tag="pA")
                    nc.tensor.matmul(ps1[:], e1w_sb[:], xtile[:], start=True, stop=True)

                    EH = ECH // 2
                    box = {}

                    def stage_a1(half):
                        if half == 0:
                            box["a1"] = encio.tile([H, ECH], BF16, tag="a1", name="a1")
                        hs = slice(half * EH, (half + 1) * EH)
                        op = nc.vector.tensor_scalar(
                            out=box["a1"][:, hs], in0=ps1[:, hs], scalar1=e1b_sb[:, 0:1],
                            scalar2=0.0, op0=ALU.add, op1=ALU.max,
                        )
                        if half == 1:
                            ps2 = auxp.tile([H, ECH], F32, tag="pB", name="ps2")
                            nc.tensor.matmul(ps2[:], e2w_bf[:], box["a1"][:], start=True, stop=True)
                            box["ps2"] = ps2
                        return op

                    def stage_ft(half):
                        hs = slice(half * EH, (half + 1) * EH)
                        fsl = slice(j * ECH + half * EH, j * ECH + (half + 1) * EH)
                        op = nc.vector.tensor_scalar(
                            out=featSB[:, fsl], in0=box["ps2"][:, hs], scalar1=e2b_sb[:, 0:1],
                            scalar2=None, op0=ALU.add,
                        )
                        if dbg and half == 1:
                            nc.sync.dma_start(featbuf[:, sl], featSB[:, sl])
                        return op

                    if slots is None:
                        # warmup/tail: ACT is idle here, use one fused op per layer
                        a1 = encio.tile([H, ECH], BF16, tag="a1")
                        nc.scalar.activation(a1[:], ps1[:], AF.Relu, bias=e1b_sb[:, 0:1])
                        ps2 = auxp.tile([H, ECH], F32, tag="pB")
                        nc.tensor.matmul(ps2[:], e2w_bf[:], a1[:], start=True, stop=True)
                        nc.vector.tensor_scalar(
                            out=featSB[:, sl], in0=ps2[:], scalar1=e2b_sb[:, 0:1],
                            scalar2=None, op0=ALU.add,
                        )
                        if dbg:
                            nc.sync.dma_start(featbuf[:, sl], featSB[:, sl])
                    else:
                        slots.extend([lambda: stage_a1(0), lambda: stage_a1(1),
                                      lambda: stage_ft(0), lambda: stage_ft(1)])

                OW = 4  # decoder chunks batched per output DMA
                owpos = {}  # side -> (tile, count, first_row)

                def dec_chunk(j, side, slots=None):
                    # immediate: h DMAs + Pool relus + psQ matmuls
                    # slot stages: q1 relu (DVE); d2 matmul + orow add (DVE)
                    sl = slice(j * ECH, (j + 1) * ECH)
                    hfr = decio.tile([H, ECH], BF16, tag="hfr")
                    nc.sync.dma_start(hfr[:], hfwbuf[:, sl])
                    hbr = decio.tile([H, ECH], BF16, tag="hbr")
                    nc.sync.dma_start(hbr[:], hbwbuf[:, sl])
                    hf_t = decio.tile([H, ECH], BF16, tag="hf")
                    hb_t = decio.tile([H, ECH], BF16, tag="hb")
                    if slots is None:
                        nc.scalar.activation(hf_t[:], hfr[:], AF.Relu)
                    else:
                        nc.gpsimd.tensor_scalar(
                            out=hf_t[:], in0=hfr[:], scalar1=0.0, scalar2=None, op0=ALU.max
                        )
                    nc.gpsimd.tensor_scalar(
                        out=hb_t[:], in0=hbr[:], scalar1=0.0, scalar2=None, op0=ALU.max
                    )
                    psQ = auxp.tile([H, ECH], F32, tag="pA")
                    nc.tensor.matmul(psQ[:], dw_bf[:, 0:H], hf_t[:], start=True, stop=False)
                    nc.tensor.matmul(psQ[:], dw_bf[:, H : 2 * H], hb_t[:], start=False, stop=True)

                    EH = ECH // 2
                    box = {}

                    def stage_q1(half):
                        if half == 0:
                            box["q1"] = decio.tile([H, ECH], BF16, tag="q1", name="q1")
                        hs = slice(half * EH, (half + 1) * EH)
                        op = nc.scalar.activation(
                            box["q1"][:, hs], psQ[:, hs], AF.Relu, bias=d1b_sb[:, 0:1]
                        )
                        if half == 1:
                            psR = auxp.tile([H, ECH], F32, tag="pB", name="psR")
                            nc.tensor.matmul(psR[0:1, :], d2_bf[:], box["q1"][:], start=True, stop=True)
                            box["psR"] = psR
                        return op

                    def stage_orow(half):
                        ow = owpos.get(side)
                        if ow is None:
                            owt = orowp.tile(
                                [1, OW * ECH], F32, tag=f"ow{side}", name=f"ow{side}"
                            )
                            ow = [owt, 0, j]
                            owpos[side] = ow
                        tilebuf, cnt, j0 = ow
                        pos = (j - j0) * ECH if side == "R" else (j - (j0 - (OW - 1))) * ECH
                        hs = slice(half * EH, (half + 1) * EH)
                        op = nc.vector.tensor_scalar(
                            out=tilebuf[0:1, pos + half * EH : pos + (half + 1) * EH],
                            in0=box["psR"][0:1, hs],
                            scalar1=d2b_sb[0:1, 0:1], scalar2=None, op0=ALU.add,
                        )
                        if half == 1:
                            ow[1] += 1
                            if ow[1] == OW:
                                first = j0 if side == "R" else j0 - (OW - 1)
                                r0 = first * ECH // 1024
                                nrows = OW * ECH // 1024
                                nc.sync.dma_start(out[r0 : r0 + nrows, :], tilebuf[:])
                                del owpos[side]
                        return op

                    if slots is None:
                        stage_q1(0)
                        stage_q1(1)
                        stage_orow(0)
                        stage_orow(1)
                    else:
                        def stage_q1b_then_orow():
                            # defer orow pieces until psR exists
                            op = stage_q1(1)
                            slots.extend([lambda: stage_orow(0), lambda: stage_orow(1)])
                            return op

                        aslots.extend([lambda: stage_q1(0), stage_q1b_then_orow])

                # ---- encoder warmup: both time-ends, 4 chunks each ----
                npre = min(5, nchunk // 2)
                for j in range(npre):
                    enc_chunk(j)
                    enc_chunk(nchunk - 1 - j)

                # ---------------- bidirectional LSTM recurrence ----------------
                def xstep(ch, t, close=False):
                    tt = t if ch == "f" else tsteps - 1 - t
                    g = rpsum.tile([H, 4 * NPC], F32, tag=f"G{ch}", name=f"G{ch}t")
                    for gi in range(4):
                        nc.tensor.matmul(
                            g[:, gi * NPC : (gi + 1) * NPC],
                            wx_bf[:, gi * H : (gi + 1) * H],
                            featSB[:, tt * NPC : (tt + 1) * NPC],
                            start=(gi == 0),
                            stop=close and gi == 3,
                        )
                    return g

                G = {ch: xstep(ch, 0, close=True) for ch in ("f", "b")}
                Gnext = {}
                h = {ch: None for ch in ("f", "b")}
                CP = {}  # [c | p] tile per chain
                hpar = {}
                for ch in ("f", "b"):
                    CP[ch] = state.tile([H, 2 * NPC], CDT, tag=f"CP{ch}", name=f"CP{ch}0")
                    nc.vector.memset(CP[ch][:, 0:NPC], 0.0)

                DEC0 = tsteps // 2 + 12
                slots = []  # staged enc/dec bulk-DVE thunks
                aslots = []  # staged dec ACT thunks (popped in the sigob->tcb window)
                for t in range(tsteps):
                    # PE: h-parts for step t, then x-parts for step t+1
                    for ch in ("f", "b"):
                        if h[ch] is not None:
                            for gi in range(4):
                                nc.tensor.matmul(
                                    G[ch][:, gi * NPC : (gi + 1) * NPC],
                                    u_bf[:, gi * H : (gi + 1) * H],
                                    h[ch][:],
                                    start=False,
                                    stop=(gi == 3),
                                    skip_group_check=True,
                                )
                        if t + 1 < tsteps:
                            Gnext[ch] = xstep(ch, t + 1)
                    # elementwise, forced same-engine ring order tuned so the
                    # staggered chains hide each other's serial latency:
                    #   ACT: sig3f sigof sig3b tcf sigob tcb
                    #   DVE: pf mm12f addf pb mm12b hnf addb hnb
                    acts = []
                    ves = []
                    S = {}
                    CPn = {}
                    M = {}
                    tcn = {}
                    hns = {}

                    def sig3(ch):
                        S[ch] = relem.tile([H, 4 * NPC], BF16, tag=f"S{ch}", name=f"S{ch}t")
                        acts.append(
                            nc.scalar.activation(
                                S[ch][:, 0 : 3 * NPC], G[ch][:, 0 : 3 * NPC], AF.Sigmoid
                            )
                        )

                    def sigo(ch):
                        acts.append(
                            nc.scalar.activation(
                                S[ch][:, 3 * NPC : 4 * NPC],
                                G[ch][:, 3 * NPC : 4 * NPC],
                                AF.Sigmoid,
                            )
                        )

                    def cblock(ch):
                        # p = 2*s_g - 1 into CP right half (4x TensorScalar)
                        ves.append(
                            nc.vector.tensor_scalar(
                                out=CP[ch][:, NPC : 2 * NPC],
                                in0=S[ch][:, 2 * NPC : 3 * NPC],
                                scalar1=2.0,
                                scalar2=-1.0,
                                op0=ALU.mult,
                                op1=ALU.add,
                            )
                        )
                        M[ch] = relem.tile([H, 2 * NPC], CDT, tag=f"M{ch}", name=f"M{ch}t")
                        ves.append(
                            nc.vector.tensor_mul(M[ch][:], S[ch][:, 0 : 2 * NPC], CP[ch][:])
                        )

                    def cadd(ch):
                        CPn[ch] = state.tile([H, 2 * NPC], CDT, tag=f"CP{ch}", name=f"CP{ch}n")
                        ves.append(
                            nc.vector.tensor_add(
                                CPn[ch][:, 0:NPC], M[ch][:, 0:NPC], M[ch][:, NPC : 2 * NPC]
                            )
                        )

                    def tanhc(ch):
                        tcn[ch] = relem.tile([H, NPC], CDT, tag=f"tc{ch}", name=f"tc{ch}t")
                        acts.append(nc.scalar.activation(tcn[ch][:], CPn[ch][:, 0:NPC], AF.Tanh))

                    def hmul(ch):
                        if t % HG == 0:
                            hpar[ch] = state.tile(
                                [H, HG * NPC], BF16, tag=f"h{ch}", name=f"h{ch}n"
                            )
                        sb = t % HG if ch == "f" else HG - 1 - (t % HG)
                        hn = hpar[ch][:, sb * NPC : (sb + 1) * NPC]
                        hns[ch] = hn
                        ves.append(
                            nc.vector.tensor_mul(hn, S[ch][:, 3 * NPC : 4 * NPC], tcn[ch][:])
                        )

                    A, B = ("f", "b") if t % 2 == 0 or not ALT else ("b", "f")
                    sig3(A)
                    sigo(A)
                    sig3(B)
                    cblock(A)
                    cadd(A)
                    tanhc(A)
                    cblock(B)
                    sigo(B)
                    hmul(A)
                    cadd(B)
                    if slots:
                        op = slots.pop(0)()  # bulk piece in the addb->hnb DVE gap
                        tile.add_dep_helper(op.ins, ves[-1].ins, sync=False, reason="slot pin")
                    if aslots:
                        op = aslots.pop(0)()  # dec ACT piece in the sigob->tcb window
                        tile.add_dep_helper(op.ins, acts[-1].ins, sync=False, reason="slot pin")
                    tanhc(B)
                    hmul(B)
                    if slots:
                        op = slots.pop(0)()  # second piece in the post-hnb DVE gap
                        tile.add_dep_helper(op.ins, ves[-1].ins, sync=False, reason="slot pin")
                    for ch in ("f", "b"):
                        if t % HG == HG - 1:
                            dst = hfwbuf if ch == "f" else hbwbuf
                            lo = (t - HG + 1) if ch == "f" else (tsteps - 1 - t)
                            nc.sync.dma_start(dst[:, lo * NPC : (lo + HG) * NPC], hpar[ch][:])
                        h[ch] = hns[ch]
                        CP[ch] = CPn[ch]
                    rsrc = {"both": (acts, ves), "acts": (acts,), "ves": (ves,), "none": ()}[RING]
                    for seq in rsrc:
                        ops = [o for o in seq if o is not acts[1]] if (seq is acts and FLOATO) else seq
                        for a, b2 in zip(ops, ops[1:]):
                            tile.add_dep_helper(b2.ins, a.ins, sync=False, reason="ring order")
                    G = Gnext
                    Gnext = {}
                    # interleaved encoder (first half) / decoder (second half)
                    if t % 4 == 0:
                        k = t // 4
                        if npre + k < nchunk // 2:
                            enc_chunk(npre + k, slots)
                            enc_chunk(nchunk - 1 - npre - k, slots)
                    if t >= DEC0 and t % 2 == 0:
                        m = (t - DEC0) // 4
                        if nchunk // 2 + m < nchunk - 4:
                            if t % 4 == 0:
                                dec_chunk(nchunk // 2 + m, "R", slots)
                            else:
                                dec_chunk(nchunk // 2 - 1 - m, "L", slots)

                for th in aslots + slots:
                    th()
                slots = []
                aslots = []

                # ---- decoder tail: remaining chunks ----
                done = set()
                if DEC0 % 4 != 0:
                    DEC0 += 4 - DEC0 % 4
                for t in range(DEC0, tsteps, 4):
                    m = (t - DEC0) // 4
                    if nchunk // 2 + m < nchunk - 4:
                        done.add(nchunk // 2 + m)
                        done.add(nchunk // 2 - 1 - m)
                rest = sorted(set(range(nchunk)) - done)
                # emit leftovers from both ends inward, in OW-aligned groups
                lo_side = sorted([j for j in rest if j < nchunk // 2], reverse=True)
                hi_side = sorted([j for j in rest if j >= nchunk // 2])
                for jl, jr in zip(lo_side, hi_side):
                    dec_chunk(jl, "L")
                    dec_chunk(jr, "R")
                for j in lo_side[len(hi_side):]:
                    dec_chunk(j, "L")
                for j in hi_side[len(lo_side):]:
                    dec_chunk(j, "R")

    nc.finalize()
    return nc


def _prep_shared(inputs):
    w_ih, w_hh = inputs["w_ih"], inputs["w_hh"]
    blocks = lambda w: (w[0:H], w[H : 2 * H], w[2 * H : 3 * H], w[3 * H : 4 * H])
    Wi, Wf, Wg, Wo = blocks(w_ih)
    Ui, Uf, Ug, Uo = blocks(w_hh)
    f32 = lambda a: np.ascontiguousarray(a, dtype=np.float32)
    shared = {
        # gate order f, i, g, o; g-gate weights doubled so
        # tanh(g) = 2*sigmoid(2g) - 1
        "wstack": f32(
            np.concatenate(
                [Wf.T, Wi.T, 2.0 * Wg.T, Wo.T, Uf.T, Ui.T, 2.0 * Ug.T, Uo.T], axis=1
            )
        ),
        "e1wT": f32(inputs["enc1_w"].T),
        "e1b": f32(inputs["enc1_b"][:, None]),
        "e2wT": f32(inputs["enc2_w"].T),
        "e2b": f32(inputs["enc2_b"][:, None]),
        "dwT": f32(
            np.concatenate([inputs["dec1_w"][:, :H].T, inputs["dec1_w"][:, H:].T], axis=1)
        ),
        "d1b": f32(inputs["dec1_b"][:, None]),
        "d2T": f32(inputs["dec2_w"].T),
        "d2b": f32(inputs["dec2_b"][:, None]),
    }
    return shared


_NC_CACHE = None


def _get_nc():
    global _NC_CACHE
    if _NC_CACHE is None:
        _NC_CACHE = build_program(T)
    return _NC_CACHE


def kernel(**inputs) -> np.ndarray:
    global LAST_RESULTS
    inputs = {k: np.asarray(v) for k, v in inputs.items()}
    x = inputs["x"]

    nc = _get_nc()
    shared = _prep_shared(inputs)

    in_maps = []
    for cidx in range(NCORES):
        xc = x[cidx * NPC : (cidx + 1) * NPC]  # (128, T, 2)
        xtc = np.ascontiguousarray(
            xc.transpose(2, 1, 0).reshape(P, T * NPC), dtype=np.float32
        )
        m = {"xt": xtc}
        m.update(shared)
        in_maps.append(m)

    res = run_bass_kernel_spmd(nc, in_maps, core_ids=list(range(NCORES)))
    LAST_RESULTS = res

    outs = []
    for cidx in range(NCORES):
        o = res.results[cidx]["out"]  # [T//8, 1024]; o[j, k*128+n] = q[n, 8j+k]
        q = o.reshape(T // 8, 8, NPC).transpose(2, 0, 1).reshape(NPC, T)
        outs.append(q)
    return np.ascontiguousarray(np.concatenate(outs, axis=0), dtype=np.float32)


if __name__ == "__main__":
    print("building program...")
    nc = build_program(32)
    print("ok, instructions:", sum(len(bb.instructions) for bb in nc.main_func.blocks))
